# revision 1
# baseline (speedup 1.0000x reference)
"""KAN layer (histogram binning) Trainium2 kernel — transposeless bf16 design.

Math reformulation (exact for linear interpolation on a uniform grid, hat
basis):
  proj = clip(x @ P, +-0.99)                          [N, 3]
  out  = tanh(sum_r w'[n, r] * cpbn[r, :])
  where, per (grid g, component c) row r = g*C + c:
    w'[n, r]   = min(|proj[n, c] - grid[g]| - 0.4, 0)   (= -relu(0.4 - |d|))
    cpbn[r, :] = -2.5 * control_points[c, g, :] * component_weights[c]
  (relu(1 - 2.5|d|) = 2.5 * relu(0.4 - |d|); both minus signs cancel in the
  matmul, so no extra negate instruction is needed.)

Layout: the host uploads x pre-transposed per core ([D, tok] fp16), so the
kernel needs NO PE transpose and no PSUM->SBUF staging copy for x. Per
512-token quarter:
  u = qmat^T @ xT (2 accumulating fp16 matmuls over the two 128-feature
  halves) -> clip (DVE min/max) -> |u - g| (ACT Abs with per-partition bias;
  the DVE TensorScalarPtr form rejects abs_max at ISA level) -> w' (DVE
  subtract/min, bf16 out) -> out = w'^T @ cpbn per 128-token chunk (bf16
  matmul) -> tanh (ACT, [128, 1024] per instruction, bf16 out) -> DMA out
  (p-major dram layout: one contiguous 8 KiB segment per partition per
  supertile).

HBM traffic per core: 4 MiB fp16 in + 4 MiB bf16 out (~23 us at 358 GB/s);
measured steady-state ~45 us/pass. fp16 x/P + bf16 w/cpbn/out keeps rel err
~7e-3 vs the 2e-2 gate (bf16 x alone would be 2.6e-2 — too lossy).

Host side: shard tokens across 8 cores, upload xT fp16, download bf16
p-major output, reorder + upcast to f32.
"""

from contextlib import ExitStack, nullcontext

import numpy as np

import concourse.bass as bass
import concourse.bacc as bacc
import concourse.tile as tile
from concourse import mybir
from concourse.bass_utils import run_bass_kernel_spmd

N_CORES = 8
TOK_TOTAL = 32 * 2048
D = 256
O = 256
G = 6
C = 3
R = G * C  # 18 hat-basis rows
SUPER = 2048  # tokens per supertile
QUART = 512  # tokens per PSUM-bank-sized quarter
CHUNK = 128  # tokens per output matmul (partition dim)

F32 = mybir.dt.float32
F32R = mybir.dt.float32r
F16 = mybir.dt.float16
BF16 = mybir.dt.bfloat16
BF16_NP = mybir.dt.np(mybir.dt.bfloat16)


def build_nc(tok_per_core: int, repeat: int = 1, n_cores: int = N_CORES):
    """Build the per-core kernel. `repeat` wraps the whole body in a hardware
    For_i loop (used only by benchmarking to amortize dispatch overhead)."""
    n_super = tok_per_core // SUPER
    assert tok_per_core % SUPER == 0

    nc = bacc.Bacc(
        "TRN2", target_bir_lowering=False, debug=False, num_devices=n_cores
    )
    xt_d = nc.dram_tensor("xt", [D, tok_per_core], F16, kind="ExternalInput").ap()
    q_d = nc.dram_tensor("qmat", [128, 2 * R], F16, kind="ExternalInput").ap()
    cp_d = nc.dram_tensor("cpbn", [32, O], BF16, kind="ExternalInput").ap()
    b_d = nc.dram_tensor("biasp", [32, 1], F32, kind="ExternalInput").ap()
    out_d = nc.dram_tensor(
        "out", [n_super * 128, (SUPER // CHUNK) * O], BF16, kind="ExternalOutput"
    ).ap()

    # xt[(h*128+p), (s*SUPER+t)] -> [s, p, h, t]: per partition, two
    # contiguous 2*SUPER-byte segments per supertile.
    x_v = xt_d.rearrange("(h p) (s t) -> s p h t", p=128, t=SUPER)
    # out[(s*128+p), (j*O+o)]: one contiguous 8 KiB segment per partition
    # per supertile.
    o_v = out_d.rearrange("(s p) f -> s p f", p=128)

    with tile.TileContext(nc) as tc, ExitStack() as ctx:
        const_p = ctx.enter_context(tc.tile_pool(name="const", bufs=1))
        xn_p = ctx.enter_context(tc.tile_pool(name="xn", bufs=2))
        ups_p = ctx.enter_context(tc.tile_pool(name="ups", bufs=2, space="PSUM"))
        w_p = ctx.enter_context(tc.tile_pool(name="w", bufs=2))
        ops_p = ctx.enter_context(tc.tile_pool(name="ops", bufs=2, space="PSUM"))
        osb_p = ctx.enter_context(tc.tile_pool(name="osb", bufs=2))

        qmat = const_p.tile([128, 2 * R], F16)
        nc.sync.dma_start(qmat[:], q_d)
        cpbn = const_p.tile([32, O], BF16)
        nc.sync.dma_start(cpbn[:], cp_d)
        biasp = const_p.tile([32, 1], F32)
        nc.sync.dma_start(biasp[:], b_d)

        loop_cm = tc.For_i(0, repeat, 1) if repeat > 1 else nullcontext()
        with loop_cm:
            for s in range(n_super):
                xt = xn_p.tile([128, 2 * SUPER], F16)
                nc.sync.dma_start(
                    xt.rearrange("p (h t) -> p h t", h=2), x_v[s]
                )
                out_sb = osb_p.tile([128, (SUPER // CHUNK) * O], BF16)
                for q in range(SUPER // QUART):
                    u_ps = ups_p.tile([128, QUART], F32, tag="ups")
                    nc.tensor.matmul(
                        u_ps[0:R, :],
                        qmat[:, 0:R],
                        xt[:, QUART * q : QUART * (q + 1)],
                        start=True,
                        stop=False,
                    )
                    nc.tensor.matmul(
                        u_ps[0:R, :],
                        qmat[:, R : 2 * R],
                        xt[:, SUPER + QUART * q : SUPER + QUART * (q + 1)],
                        start=False,
                        stop=True,
                    )
                    u_sb = w_p.tile([32, QUART], F32, tag="usb")
                    nc.vector.tensor_scalar(
                        u_sb[0:R, :],
                        u_ps[0:R, :],
                        0.99,
                        -0.99,
                        op0=mybir.AluOpType.min,
                        op1=mybir.AluOpType.max,
                    )
                    a_sb = w_p.tile([32, QUART], F32, tag="asb")
                    nc.scalar.activation(
                        a_sb[0:R, :],
                        u_sb[0:R, :],
                        mybir.ActivationFunctionType.Abs,
                        bias=biasp[0:R, :],
                        scale=1.0,
                    )
                    w_sb = w_p.tile([32, QUART], BF16, tag="wsb")
                    nc.vector.tensor_scalar(
                        w_sb[0:R, :],
                        a_sb[0:R, :],
                        0.4,
                        0.0,
                        op0=mybir.AluOpType.subtract,
                        op1=mybir.AluOpType.min,
                    )
                    out_ps = ops_p.tile([128, 4 * O], F32, tag="ops")
                    for k in range(QUART // CHUNK):
                        nc.tensor.matmul(
                            out_ps[:, O * k : O * (k + 1)],
                            w_sb[0:R, CHUNK * k : CHUNK * (k + 1)],
                            cpbn[0:R, :],
                            start=True,
                            stop=True,
                        )
                    nc.scalar.activation(
                        out_sb[:, 4 * O * q : 4 * O * (q + 1)],
                        out_ps[:],
                        mybir.ActivationFunctionType.Tanh,
                    )
                nc.sync.dma_start(o_v[s], out_sb[:])

    nc.compile()
    return nc


def make_consts(projections: np.ndarray, control_points: np.ndarray,
                component_weights: np.ndarray):
    grid = np.linspace(-1.0, 1.0, G).astype(np.float32)
    qmat = np.zeros((128, 2 * R), np.float32)
    for h in range(2):
        for r in range(R):
            qmat[:, h * R + r] = projections[h * 128 : (h + 1) * 128, r % C]
    cpbn = np.zeros((32, O), np.float32)
    biasp = np.zeros((32, 1), np.float32)
    for r in range(R):
        g, c = r // C, r % C
        cpbn[r] = -2.5 * control_points[c, g] * component_weights[c]
        biasp[r, 0] = -grid[g]
    return qmat.astype(np.float16), cpbn.astype(BF16_NP), biasp


def make_in_maps(x: np.ndarray, projections: np.ndarray,
                 control_points: np.ndarray, component_weights: np.ndarray):
    tok = x.shape[0] * x.shape[1]
    tok_per_core = tok // N_CORES
    qmat, cpbn, biasp = make_consts(
        np.asarray(projections), np.asarray(control_points),
        np.asarray(component_weights)
    )
    flat = x.reshape(tok, D)
    in_maps = []
    for i in range(N_CORES):
        xt = flat[i * tok_per_core : (i + 1) * tok_per_core].T.astype(np.float16)
        in_maps.append(
            {"xt": np.ascontiguousarray(xt), "qmat": qmat, "cpbn": cpbn,
             "biasp": biasp}
        )
    return in_maps


def assemble_output(outs, B, S):
    """outs: per-core [n_super*128, 16*O] bf16 p-major arrays -> [B, S, O] f32."""
    tok_per_core = TOK_TOTAL // N_CORES
    n_super = tok_per_core // SUPER
    parts = []
    for o in outs:
        a = np.asarray(o).reshape(n_super, 128, SUPER // CHUNK, O)
        parts.append(
            a.transpose(0, 2, 1, 3).reshape(tok_per_core, O).astype(np.float32)
        )
    return np.concatenate(parts, axis=0).reshape(B, S, O)


_NC_CACHE = {}


def kernel(x, projections, control_points, component_weights, _trace=False):
    x = np.asarray(x)
    B, S, _ = x.shape
    tok = B * S
    tok_per_core = tok // N_CORES
    key = (tok_per_core, 1)
    if key not in _NC_CACHE:
        _NC_CACHE[key] = build_nc(tok_per_core)
    nc = _NC_CACHE[key]

    in_maps = make_in_maps(x, projections, control_points, component_weights)
    res = run_bass_kernel_spmd(nc, in_maps, list(range(N_CORES)), trace=_trace)
    ret = assemble_output(
        [res.results[i]["out"] for i in range(N_CORES)], B, S
    ).astype(x.dtype, copy=False)
    if _trace:
        return ret, res
    return ret



# revision 4
# speedup vs baseline: 1.0536x; 1.0536x over previous
"""KAN layer (histogram binning) Trainium2 kernel — transposeless bf16 design,
v2: partition-packed elementwise chain.

Math reformulation (exact for linear interpolation on a uniform grid, hat
basis):
  proj = clip(x @ P, +-0.99)                          [N, 3]
  out  = tanh(sum_r w'[n, r] * cpbn[r, :])
  where, per (grid g, component c) row r = g*C + c:
    w'[n, r]   = min(|proj[n, c] - grid[g]| - 0.4, 0)   (= -relu(0.4 - |d|))
    cpbn[r, :] = -2.5 * control_points[c, g, :] * component_weights[c]
  (relu(1 - 2.5|d|) = 2.5 * relu(0.4 - |d|); both minus signs cancel in the
  matmul, so no extra negate instruction is needed.)

v2 layout change vs v1: the 18 hat rows of all FOUR 512-token quarters of a
supertile are packed into ONE [128, 512] tile at 32-aligned partition offsets
(quarter j -> partitions 32j..32j+17; rows 18..31 of each group are a zero
pad kept PE-written via 14 zero columns in qmat).  The clip/abs/w elementwise
chain then runs once per 2048-token supertile instead of once per quarter —
4x fewer DVE/ACT instructions, since their cost is free-dim driven and was
wasting 110/128 lanes in v1.  tile_position for all matmuls auto-derives from
the 32-aligned partition bases (bass requires lhsT.base == rhs.base, and col
groups at 0/32/64/96).

Per 2048-token supertile:
  DMA in xt [128, 4096] fp16 (one contiguous 8 KiB segment per partition)
  8 matmuls (4 quarters x 2 feature halves) -> u_ps [128, 512] PSUM
  DVE clip (min/max)            [128, 512] psum f32 -> sbuf f16
  ACT Abs with per-partition -g bias       f16 -> f16
  DVE (sub 0.4, min 0)                     f16 -> bf16
  16 matmuls (4 quarters x 4 token chunks): w^T @ cpbn -> out_ps [128, 1024]
  4x ACT Tanh [128, 1024] psum f32 -> out_sb bf16
  DMA out [128, 4096] bf16 (one contiguous 8 KiB segment per partition)

HBM traffic per core: 4 MiB fp16 in + 4 MiB bf16 out (~23.5 us at 358 GB/s).
fp16 x/P + bf16 w/cpbn/out keeps rel err ~7e-3 vs the 2e-2 gate (bf16 x
alone would be 2.6e-2 — too lossy).

Host side: shard tokens across 8 cores, upload xT fp16 (supertile-major,
8 KiB contiguous per partition), download bf16 p-major output, reorder +
upcast to f32.
"""

from contextlib import ExitStack, nullcontext

import numpy as np

import concourse.bass as bass
import concourse.bacc as bacc
import concourse.tile as tile
from concourse import mybir
from concourse.bass_utils import run_bass_kernel_spmd

N_CORES = 8
TOK_TOTAL = 32 * 2048
D = 256
O = 256
G = 6
C = 3
R = G * C  # 18 hat-basis rows
RP = 32  # padded row group stride (rows r>=R are a zero pad)
SUPER = 2048  # tokens per supertile
QUART = 512  # tokens per PSUM-bank-sized quarter
CHUNK = 128  # tokens per output matmul (partition dim)

F32 = mybir.dt.float32
F16 = mybir.dt.float16
BF16 = mybir.dt.bfloat16
BF16_NP = mybir.dt.np(mybir.dt.bfloat16)


def build_nc(tok_per_core: int, repeat: int = 1, n_cores: int = N_CORES):
    """Build the per-core kernel. `repeat` wraps the whole body in a hardware
    For_i loop (used only by benchmarking to amortize dispatch overhead)."""
    n_super = tok_per_core // SUPER
    assert tok_per_core % SUPER == 0

    nc = bacc.Bacc(
        "TRN2", target_bir_lowering=False, debug=False, num_devices=n_cores
    )
    xt_d = nc.dram_tensor(
        "xt", [n_super * 128, 2 * SUPER], F16, kind="ExternalInput"
    ).ap()
    q_d = nc.dram_tensor("qmat", [128, 2 * RP], F16, kind="ExternalInput").ap()
    cp_d = nc.dram_tensor("cpbn", [128, O], BF16, kind="ExternalInput").ap()
    b_d = nc.dram_tensor("biasp", [128, 1], F32, kind="ExternalInput").ap()
    out_d = nc.dram_tensor(
        "out", [n_super * 128, (SUPER // CHUNK) * O], BF16, kind="ExternalOutput"
    ).ap()

    # xt[(s*128+p), f]: one contiguous 8 KiB segment per partition per
    # supertile.  f = h*SUPER + t (feature-half major).
    x_v = xt_d.rearrange("(s p) f -> s p f", p=128)
    # out[(s*128+p), (j*O+o)]: one contiguous 8 KiB segment per partition
    # per supertile.
    o_v = out_d.rearrange("(s p) f -> s p f", p=128)

    with tile.TileContext(nc) as tc, ExitStack() as ctx:
        const_p = ctx.enter_context(tc.tile_pool(name="const", bufs=1))
        xn_p = ctx.enter_context(tc.tile_pool(name="xn", bufs=2))
        ups_p = ctx.enter_context(tc.tile_pool(name="ups", bufs=2, space="PSUM"))
        w_p = ctx.enter_context(tc.tile_pool(name="w", bufs=2))
        ops_p = ctx.enter_context(tc.tile_pool(name="ops", bufs=2, space="PSUM"))
        osb_p = ctx.enter_context(tc.tile_pool(name="osb", bufs=2))

        qmat = const_p.tile([128, 2 * RP], F16)
        nc.sync.dma_start(qmat[:], q_d)
        cpbn = const_p.tile([128, O], BF16)
        nc.sync.dma_start(cpbn[:], cp_d)
        biasp = const_p.tile([128, 1], F32)
        nc.sync.dma_start(biasp[:], b_d)

        loop_cm = tc.For_i(0, repeat, 1) if repeat > 1 else nullcontext()
        with loop_cm:
            for s in range(n_super):
                xt = xn_p.tile([128, 2 * SUPER], F16)
                nc.sync.dma_start(xt[:], x_v[s])
                out_sb = osb_p.tile([128, (SUPER // CHUNK) * O], BF16)
                u_ps = ups_p.tile([128, QUART], F32, tag="ups")
                for j in range(SUPER // QUART):
                    nc.tensor.matmul(
                        u_ps[RP * j : RP * (j + 1), :],
                        qmat[:, 0:RP],
                        xt[:, QUART * j : QUART * (j + 1)],
                        start=True,
                        stop=False,
                        tile_position=(0, RP * j),
                    )
                    nc.tensor.matmul(
                        u_ps[RP * j : RP * (j + 1), :],
                        qmat[:, RP : 2 * RP],
                        xt[:, SUPER + QUART * j : SUPER + QUART * (j + 1)],
                        start=False,
                        stop=True,
                        tile_position=(0, RP * j),
                    )
                u_sb = w_p.tile([128, QUART], F16, tag="usb")
                nc.vector.tensor_scalar(
                    u_sb[:],
                    u_ps[:],
                    0.99,
                    -0.99,
                    op0=mybir.AluOpType.min,
                    op1=mybir.AluOpType.max,
                )
                a_sb = w_p.tile([128, QUART], F16, tag="asb")
                nc.scalar.activation(
                    a_sb[:],
                    u_sb[:],
                    mybir.ActivationFunctionType.Abs,
                    bias=biasp[:],
                    scale=1.0,
                )
                w_sb = w_p.tile([128, QUART], BF16, tag="wsb")
                nc.vector.tensor_scalar(
                    w_sb[:],
                    a_sb[:],
                    0.4,
                    0.0,
                    op0=mybir.AluOpType.subtract,
                    op1=mybir.AluOpType.min,
                )
                for j in range(SUPER // QUART):
                    out_ps = ops_p.tile([128, 4 * O], F32, tag="ops")
                    for k in range(QUART // CHUNK):
                        nc.tensor.matmul(
                            out_ps[:, O * k : O * (k + 1)],
                            w_sb[RP * j : RP * j + R, CHUNK * k : CHUNK * (k + 1)],
                            cpbn[RP * j : RP * j + R, :],
                            start=True,
                            stop=True,
                            tile_position=(RP * j, 0),
                        )
                    nc.scalar.activation(
                        out_sb[:, 4 * O * j : 4 * O * (j + 1)],
                        out_ps[:],
                        mybir.ActivationFunctionType.Tanh,
                    )
                nc.sync.dma_start(o_v[s], out_sb[:])

    nc.compile()
    return nc


def make_consts(projections: np.ndarray, control_points: np.ndarray,
                component_weights: np.ndarray):
    grid = np.linspace(-1.0, 1.0, G).astype(np.float32)
    qmat = np.zeros((128, 2 * RP), np.float32)
    for h in range(2):
        for r in range(R):
            qmat[:, h * RP + r] = projections[h * 128 : (h + 1) * 128, r % C]
    cpbn = np.zeros((128, O), np.float32)
    biasp = np.zeros((128, 1), np.float32)
    for j in range(SUPER // QUART):
        for r in range(R):
            g, c = r // C, r % C
            cpbn[RP * j + r] = -2.5 * control_points[c, g] * component_weights[c]
            biasp[RP * j + r, 0] = -grid[g]
    return qmat.astype(np.float16), cpbn.astype(BF16_NP), biasp


def make_in_maps(x: np.ndarray, projections: np.ndarray,
                 control_points: np.ndarray, component_weights: np.ndarray):
    tok = x.shape[0] * x.shape[1]
    tok_per_core = tok // N_CORES
    n_super = tok_per_core // SUPER
    qmat, cpbn, biasp = make_consts(
        np.asarray(projections), np.asarray(control_points),
        np.asarray(component_weights)
    )
    flat = x.reshape(tok, D)
    in_maps = []
    for i in range(N_CORES):
        xc = flat[i * tok_per_core : (i + 1) * tok_per_core]
        # [tok, D] -> [s, p, h, t] -> [(s p), (h t)]: per supertile s, each
        # partition p holds feature h*128+p for tokens t, 8 KiB contiguous.
        xt = (
            xc.reshape(n_super, SUPER, 2, 128)
            .transpose(0, 3, 2, 1)
            .reshape(n_super * 128, 2 * SUPER)
            .astype(np.float16)
        )
        in_maps.append(
            {"xt": np.ascontiguousarray(xt), "qmat": qmat, "cpbn": cpbn,
             "biasp": biasp}
        )
    return in_maps


def assemble_output(outs, B, S):
    """outs: per-core [n_super*128, 16*O] bf16 p-major arrays -> [B, S, O] f32."""
    tok_per_core = TOK_TOTAL // N_CORES
    n_super = tok_per_core // SUPER
    parts = []
    for o in outs:
        a = np.asarray(o).reshape(n_super, 128, SUPER // CHUNK, O)
        parts.append(
            a.transpose(0, 2, 1, 3).reshape(tok_per_core, O).astype(np.float32)
        )
    return np.concatenate(parts, axis=0).reshape(B, S, O)


_NC_CACHE = {}


def kernel(x, projections, control_points, component_weights, _trace=False):
    x = np.asarray(x)
    B, S, _ = x.shape
    tok = B * S
    tok_per_core = tok // N_CORES
    key = (tok_per_core, 1)
    if key not in _NC_CACHE:
        _NC_CACHE[key] = build_nc(tok_per_core)
    nc = _NC_CACHE[key]

    in_maps = make_in_maps(x, projections, control_points, component_weights)
    res = run_bass_kernel_spmd(nc, in_maps, list(range(N_CORES)), trace=_trace)
    ret = assemble_output(
        [res.results[i]["out"] for i in range(N_CORES)], B, S
    ).astype(x.dtype, copy=False)
    if _trace:
        return ret, res
    return ret


# revision 5
# speedup vs baseline: 1.5933x; 1.5122x over previous
"""KAN layer (histogram binning) Trainium2 kernel — transposeless bf16 design,
v2: partition-packed elementwise chain.

Math reformulation (exact for linear interpolation on a uniform grid, hat
basis):
  proj = clip(x @ P, +-0.99)                          [N, 3]
  out  = tanh(sum_r w'[n, r] * cpbn[r, :])
  where, per (grid g, component c) row r = g*C + c:
    w'[n, r]   = min(|proj[n, c] - grid[g]| - 0.4, 0)   (= -relu(0.4 - |d|))
    cpbn[r, :] = -2.5 * control_points[c, g, :] * component_weights[c]
  (relu(1 - 2.5|d|) = 2.5 * relu(0.4 - |d|); both minus signs cancel in the
  matmul, so no extra negate instruction is needed.)

v2 layout change vs v1: the 18 hat rows of all FOUR 512-token quarters of a
supertile are packed into ONE [128, 512] tile at 32-aligned partition offsets
(quarter j -> partitions 32j..32j+17; rows 18..31 of each group are a zero
pad kept PE-written via 14 zero columns in qmat).  The clip/abs/w elementwise
chain then runs once per 2048-token supertile instead of once per quarter —
4x fewer DVE/ACT instructions, since their cost is free-dim driven and was
wasting 110/128 lanes in v1.  tile_position for all matmuls auto-derives from
the 32-aligned partition bases (bass requires lhsT.base == rhs.base, and col
groups at 0/32/64/96).

Per 2048-token supertile:
  DMA in xt [128, 4096] fp16 (one contiguous 8 KiB segment per partition)
  8 matmuls (4 quarters x 2 feature halves) -> u_ps [128, 512] PSUM
  DVE clip (min/max)            [128, 512] psum f32 -> sbuf f16
  ACT Abs with per-partition -g bias       f16 -> f16
  DVE (sub 0.4, min 0)                     f16 -> bf16
  16 matmuls (4 quarters x 4 token chunks): w^T @ cpbn -> out_ps [128, 1024]
  4x ACT Tanh [128, 1024] psum f32 -> out_sb bf16
  DMA out [128, 4096] bf16 (one contiguous 8 KiB segment per partition)

HBM traffic per core: 4 MiB fp16 in + 4 MiB bf16 out (~23.5 us at 358 GB/s).
fp16 x/P + bf16 w/cpbn/out keeps rel err ~7e-3 vs the 2e-2 gate (bf16 x
alone would be 2.6e-2 — too lossy).

Host side: shard tokens across 8 cores, upload xT fp16 (supertile-major,
8 KiB contiguous per partition), download bf16 p-major output, reorder +
upcast to f32.
"""

from contextlib import ExitStack, nullcontext

import numpy as np

import concourse.bass as bass
import concourse.bacc as bacc
import concourse.tile as tile
from concourse import mybir
from concourse.bass_utils import run_bass_kernel_spmd

N_CORES = 8
TOK_TOTAL = 32 * 2048
D = 256
O = 256
G = 6
C = 3
R = G * C  # 18 hat-basis rows
RP = 32  # padded row group stride (rows r>=R are a zero pad)
SUPER = 2048  # tokens per supertile
QUART = 512  # tokens per PSUM-bank-sized quarter
CHUNK = 128  # tokens per output matmul (partition dim)

F32 = mybir.dt.float32
F16 = mybir.dt.float16
BF16 = mybir.dt.bfloat16
BF16_NP = mybir.dt.np(mybir.dt.bfloat16)


def build_nc(tok_per_core: int, repeat: int = 1, n_cores: int = N_CORES):
    """Build the per-core kernel. `repeat` wraps the whole body in a hardware
    For_i loop (used only by benchmarking to amortize dispatch overhead)."""
    n_super = tok_per_core // SUPER
    assert tok_per_core % SUPER == 0

    nc = bacc.Bacc(
        "TRN2", target_bir_lowering=False, debug=False, num_devices=n_cores
    )
    xt_d = nc.dram_tensor(
        "xt", [n_super * 128, 2 * SUPER], F16, kind="ExternalInput"
    ).ap()
    q_d = nc.dram_tensor("qmat", [128, 2 * RP], F16, kind="ExternalInput").ap()
    cp_d = nc.dram_tensor("cpbn", [128, O], BF16, kind="ExternalInput").ap()
    b_d = nc.dram_tensor("biasp", [128, 1], F32, kind="ExternalInput").ap()
    out_d = nc.dram_tensor(
        "out", [n_super * 128, (SUPER // CHUNK) * O], BF16, kind="ExternalOutput"
    ).ap()

    # xt[(s*128+p), f]: one contiguous 8 KiB segment per partition per
    # supertile.  f = h*SUPER + t (feature-half major).
    x_v = xt_d.rearrange("(s p) f -> s p f", p=128)
    # out[(s*128+p), (j*O+o)]: one contiguous 8 KiB segment per partition
    # per supertile.
    o_v = out_d.rearrange("(s p) f -> s p f", p=128)

    # The plain For_i inserts an all-engine barrier per iteration (~9 us
    # pipeline drain).  Unroll several passes per iteration to amortize it.
    unroll = 1
    if repeat > 1:
        for cand in (8, 4, 2):
            if repeat % cand == 0:
                unroll = cand
                break

    with tile.TileContext(nc) as tc, ExitStack() as ctx:
        const_p = ctx.enter_context(tc.tile_pool(name="const", bufs=1))
        xn_p = ctx.enter_context(tc.tile_pool(name="xn", bufs=3))
        ups_p = ctx.enter_context(tc.tile_pool(name="ups", bufs=2, space="PSUM"))
        w_p = ctx.enter_context(tc.tile_pool(name="w", bufs=2))
        ops_p = ctx.enter_context(tc.tile_pool(name="ops", bufs=3, space="PSUM"))
        osb_p = ctx.enter_context(tc.tile_pool(name="osb", bufs=3))

        qmat = const_p.tile([128, 2 * RP], F16)
        nc.sync.dma_start(qmat[:], q_d)
        cpbn = const_p.tile([128, O], BF16)
        nc.sync.dma_start(cpbn[:], cp_d)
        biasp = const_p.tile([128, 1], F32)
        nc.sync.dma_start(biasp[:], b_d)

        loop_cm = (
            tc.For_i(0, repeat // unroll, 1) if repeat > 1 else nullcontext()
        )
        with loop_cm:
            for s in range(n_super * unroll):
                s = s % n_super
                xt = xn_p.tile([128, 2 * SUPER], F16)
                nc.sync.dma_start(xt[:], x_v[s])
                out_sb = osb_p.tile([128, (SUPER // CHUNK) * O], BF16)
                u_ps = ups_p.tile([128, QUART], F32, tag="ups")
                for j in range(SUPER // QUART):
                    nc.tensor.matmul(
                        u_ps[RP * j : RP * (j + 1), :],
                        qmat[:, 0:RP],
                        xt[:, QUART * j : QUART * (j + 1)],
                        start=True,
                        stop=False,
                        tile_position=(0, RP * j),
                    )
                    nc.tensor.matmul(
                        u_ps[RP * j : RP * (j + 1), :],
                        qmat[:, RP : 2 * RP],
                        xt[:, SUPER + QUART * j : SUPER + QUART * (j + 1)],
                        start=False,
                        stop=True,
                        tile_position=(0, RP * j),
                    )
                u_sb = w_p.tile([128, QUART], F16, tag="usb")
                nc.vector.tensor_scalar(
                    u_sb[:],
                    u_ps[:],
                    0.99,
                    -0.99,
                    op0=mybir.AluOpType.min,
                    op1=mybir.AluOpType.max,
                )
                a_sb = w_p.tile([128, QUART], F16, tag="asb")
                nc.scalar.activation(
                    a_sb[:],
                    u_sb[:],
                    mybir.ActivationFunctionType.Abs,
                    bias=biasp[:],
                    scale=1.0,
                )
                w_sb = w_p.tile([128, QUART], BF16, tag="wsb")
                nc.vector.tensor_scalar(
                    w_sb[:],
                    a_sb[:],
                    0.4,
                    0.0,
                    op0=mybir.AluOpType.subtract,
                    op1=mybir.AluOpType.min,
                )
                for j in range(SUPER // QUART):
                    out_ps = ops_p.tile([128, 4 * O], F32, tag="ops")
                    for k in range(QUART // CHUNK):
                        nc.tensor.matmul(
                            out_ps[:, O * k : O * (k + 1)],
                            w_sb[RP * j : RP * j + R, CHUNK * k : CHUNK * (k + 1)],
                            cpbn[RP * j : RP * j + R, :],
                            start=True,
                            stop=True,
                            tile_position=(RP * j, 0),
                        )
                    nc.scalar.activation(
                        out_sb[:, 4 * O * j : 4 * O * (j + 1)],
                        out_ps[:],
                        mybir.ActivationFunctionType.Tanh,
                    )
                nc.sync.dma_start(o_v[s], out_sb[:])

    nc.compile()
    return nc


def make_consts(projections: np.ndarray, control_points: np.ndarray,
                component_weights: np.ndarray):
    grid = np.linspace(-1.0, 1.0, G).astype(np.float32)
    qmat = np.zeros((128, 2 * RP), np.float32)
    for h in range(2):
        for r in range(R):
            qmat[:, h * RP + r] = projections[h * 128 : (h + 1) * 128, r % C]
    cpbn = np.zeros((128, O), np.float32)
    biasp = np.zeros((128, 1), np.float32)
    for j in range(SUPER // QUART):
        for r in range(R):
            g, c = r // C, r % C
            cpbn[RP * j + r] = -2.5 * control_points[c, g] * component_weights[c]
            biasp[RP * j + r, 0] = -grid[g]
    return qmat.astype(np.float16), cpbn.astype(BF16_NP), biasp


def make_in_maps(x: np.ndarray, projections: np.ndarray,
                 control_points: np.ndarray, component_weights: np.ndarray):
    tok = x.shape[0] * x.shape[1]
    tok_per_core = tok // N_CORES
    n_super = tok_per_core // SUPER
    qmat, cpbn, biasp = make_consts(
        np.asarray(projections), np.asarray(control_points),
        np.asarray(component_weights)
    )
    flat = x.reshape(tok, D)
    in_maps = []
    for i in range(N_CORES):
        xc = flat[i * tok_per_core : (i + 1) * tok_per_core]
        # [tok, D] -> [s, p, h, t] -> [(s p), (h t)]: per supertile s, each
        # partition p holds feature h*128+p for tokens t, 8 KiB contiguous.
        xt = (
            xc.reshape(n_super, SUPER, 2, 128)
            .transpose(0, 3, 2, 1)
            .reshape(n_super * 128, 2 * SUPER)
            .astype(np.float16)
        )
        in_maps.append(
            {"xt": np.ascontiguousarray(xt), "qmat": qmat, "cpbn": cpbn,
             "biasp": biasp}
        )
    return in_maps


def assemble_output(outs, B, S):
    """outs: per-core [n_super*128, 16*O] bf16 p-major arrays -> [B, S, O] f32."""
    tok_per_core = TOK_TOTAL // N_CORES
    n_super = tok_per_core // SUPER
    parts = []
    for o in outs:
        a = np.asarray(o).reshape(n_super, 128, SUPER // CHUNK, O)
        parts.append(
            a.transpose(0, 2, 1, 3).reshape(tok_per_core, O).astype(np.float32)
        )
    return np.concatenate(parts, axis=0).reshape(B, S, O)


_NC_CACHE = {}


def kernel(x, projections, control_points, component_weights, _trace=False):
    x = np.asarray(x)
    B, S, _ = x.shape
    tok = B * S
    tok_per_core = tok // N_CORES
    key = (tok_per_core, 1)
    if key not in _NC_CACHE:
        _NC_CACHE[key] = build_nc(tok_per_core)
    nc = _NC_CACHE[key]

    in_maps = make_in_maps(x, projections, control_points, component_weights)
    res = run_bass_kernel_spmd(nc, in_maps, list(range(N_CORES)), trace=_trace)
    ret = assemble_output(
        [res.results[i]["out"] for i in range(N_CORES)], B, S
    ).astype(x.dtype, copy=False)
    if _trace:
        return ret, res
    return ret


# revision 10
# speedup vs baseline: 1.7913x; 1.1243x over previous
"""KAN layer (histogram binning) Trainium2 kernel — transposeless bf16 design,
v2: partition-packed elementwise chain.

Math reformulation (exact for linear interpolation on a uniform grid, hat
basis):
  proj = clip(x @ P, +-0.99)                          [N, 3]
  out  = tanh(sum_r w'[n, r] * cpbn[r, :])
  where, per (grid g, component c) row r = g*C + c:
    w'[n, r]   = min(|proj[n, c] - grid[g]| - 0.4, 0)   (= -relu(0.4 - |d|))
    cpbn[r, :] = -2.5 * control_points[c, g, :] * component_weights[c]
  (relu(1 - 2.5|d|) = 2.5 * relu(0.4 - |d|); both minus signs cancel in the
  matmul, so no extra negate instruction is needed.)

v2 layout change vs v1: the 18 hat rows of all FOUR 512-token quarters of a
supertile are packed into ONE [128, 512] tile at 32-aligned partition offsets
(quarter j -> partitions 32j..32j+17; rows 18..31 of each group are a zero
pad kept PE-written via 14 zero columns in qmat).  The clip/abs/w elementwise
chain then runs once per 2048-token supertile instead of once per quarter —
4x fewer DVE/ACT instructions, since their cost is free-dim driven and was
wasting 110/128 lanes in v1.  tile_position for all matmuls auto-derives from
the 32-aligned partition bases (bass requires lhsT.base == rhs.base, and col
groups at 0/32/64/96).

Per 2048-token supertile:
  DMA in xt [128, 4096] fp16 (one contiguous 8 KiB segment per partition)
  8 matmuls (4 quarters x 2 feature halves) -> u_ps [128, 512] PSUM
  DVE clip (min/max)            [128, 512] psum f32 -> sbuf f16
  ACT Abs with per-partition -g bias       f16 -> f16
  DVE (sub 0.4, min 0)                     f16 -> bf16
  16 matmuls (4 quarters x 4 token chunks): w^T @ cpbn -> out_ps [128, 1024]
  4x ACT Tanh [128, 1024] psum f32 -> out_sb bf16
  DMA out [128, 4096] bf16 (one contiguous 8 KiB segment per partition)

HBM traffic per core: 4 MiB fp16 in + 4 MiB bf16 out (~23.5 us at 358 GB/s).
fp16 x/P + bf16 w/cpbn/out keeps rel err ~7e-3 vs the 2e-2 gate (bf16 x
alone would be 2.6e-2 — too lossy).

Host side: shard tokens across 8 cores, upload xT fp16 (supertile-major,
8 KiB contiguous per partition), download bf16 p-major output, reorder +
upcast to f32.
"""

from contextlib import ExitStack, nullcontext

import numpy as np

import concourse.bass as bass
import concourse.bacc as bacc
import concourse.tile as tile
from concourse import mybir
from concourse.bass_utils import run_bass_kernel_spmd

N_CORES = 8
TOK_TOTAL = 32 * 2048
D = 256
O = 256
G = 6
C = 3
R = G * C  # 18 hat-basis rows
RP = 32  # padded row group stride (rows r>=R are a zero pad)
SUPER = 2048  # tokens per supertile
QUART = 512  # tokens per PSUM-bank-sized quarter
CHUNK = 128  # tokens per output matmul (partition dim)

F32 = mybir.dt.float32
F16 = mybir.dt.float16
BF16 = mybir.dt.bfloat16
I8 = mybir.dt.int8
BF16_NP = mybir.dt.np(mybir.dt.bfloat16)
OUT_SCALE = 127.0


def build_nc(tok_per_core: int, repeat: int = 1, n_cores: int = N_CORES):
    """Build the per-core kernel. `repeat` wraps the whole body in a hardware
    For_i loop (used only by benchmarking to amortize dispatch overhead)."""
    n_super = tok_per_core // SUPER
    assert tok_per_core % SUPER == 0

    nc = bacc.Bacc(
        "TRN2", target_bir_lowering=False, debug=False, num_devices=n_cores
    )
    xt_d = nc.dram_tensor(
        "xt", [n_super * 128, 2 * SUPER], F16, kind="ExternalInput"
    ).ap()
    q_d = nc.dram_tensor("qmat", [128, 2 * RP], F16, kind="ExternalInput").ap()
    cp_d = nc.dram_tensor("cpbn", [128, O], BF16, kind="ExternalInput").ap()
    b_d = nc.dram_tensor("biasp", [128, 1], F32, kind="ExternalInput").ap()
    out_d = nc.dram_tensor(
        "out", [n_super * 128, (SUPER // CHUNK) * O], I8, kind="ExternalOutput"
    ).ap()

    # xt[(s*128+p), f]: one contiguous 8 KiB segment per partition per
    # supertile.  f = h*SUPER + t (feature-half major).
    x_v = xt_d.rearrange("(s p) f -> s p f", p=128)
    # out[(s*128+p), (j*O+o)]: one contiguous 8 KiB segment per partition
    # per supertile.
    o_v = out_d.rearrange("(s p) f -> s p f", p=128)

    # The plain For_i inserts an all-engine barrier per iteration (~9 us
    # pipeline drain).  Unroll several passes per iteration to amortize it.
    unroll = 1
    if repeat > 1:
        for cand in (8, 4, 2):
            if repeat % cand == 0:
                unroll = cand
                break

    with tile.TileContext(nc) as tc, ExitStack() as ctx:
        const_p = ctx.enter_context(tc.tile_pool(name="const", bufs=1))
        xn_p = ctx.enter_context(tc.tile_pool(name="xn", bufs=3))
        ups_p = ctx.enter_context(tc.tile_pool(name="ups", bufs=2, space="PSUM"))
        w_p = ctx.enter_context(tc.tile_pool(name="w", bufs=2))
        ops_p = ctx.enter_context(tc.tile_pool(name="ops", bufs=3, space="PSUM"))
        osb_p = ctx.enter_context(tc.tile_pool(name="osb", bufs=3))

        qmat = const_p.tile([128, 2 * RP], F16)
        nc.sync.dma_start(qmat[:], q_d)
        cpbn = const_p.tile([128, O], BF16)
        nc.sync.dma_start(cpbn[:], cp_d)
        biasp = const_p.tile([128, 1], F32)
        nc.sync.dma_start(biasp[:], b_d)

        loop_cm = (
            tc.For_i(0, repeat // unroll, 1) if repeat > 1 else nullcontext()
        )
        with loop_cm:
            for s in range(n_super * unroll):
                s = s % n_super
                xt = xn_p.tile([128, 2 * SUPER], F16)
                nc.sync.dma_start(xt[:], x_v[s])
                out_sb = osb_p.tile([128, (SUPER // CHUNK) * O], I8)
                u_ps = ups_p.tile([128, QUART], F32, tag="ups")
                for j in range(SUPER // QUART):
                    nc.tensor.matmul(
                        u_ps[RP * j : RP * (j + 1), :],
                        qmat[:, 0:RP],
                        xt[:, QUART * j : QUART * (j + 1)],
                        start=True,
                        stop=False,
                        tile_position=(0, RP * j),
                    )
                    nc.tensor.matmul(
                        u_ps[RP * j : RP * (j + 1), :],
                        qmat[:, RP : 2 * RP],
                        xt[:, SUPER + QUART * j : SUPER + QUART * (j + 1)],
                        start=False,
                        stop=True,
                        tile_position=(0, RP * j),
                    )
                u_sb = w_p.tile([128, QUART], F16, tag="usb")
                nc.vector.tensor_scalar(
                    u_sb[:],
                    u_ps[:],
                    0.99,
                    -0.99,
                    op0=mybir.AluOpType.min,
                    op1=mybir.AluOpType.max,
                )
                a_sb = w_p.tile([128, QUART], F16, tag="asb")
                nc.scalar.activation(
                    a_sb[:],
                    u_sb[:],
                    mybir.ActivationFunctionType.Abs,
                    bias=biasp[:],
                    scale=1.0,
                )
                w_sb = w_p.tile([128, QUART], BF16, tag="wsb")
                nc.vector.tensor_scalar(
                    w_sb[:],
                    a_sb[:],
                    0.4,
                    0.0,
                    op0=mybir.AluOpType.subtract,
                    op1=mybir.AluOpType.min,
                )
                for j in range(SUPER // QUART):
                    out_ps = ops_p.tile([128, 4 * O], F32, tag="ops")
                    for k in range(QUART // CHUNK):
                        nc.tensor.matmul(
                            out_ps[:, O * k : O * (k + 1)],
                            w_sb[RP * j : RP * j + R, CHUNK * k : CHUNK * (k + 1)],
                            cpbn[RP * j : RP * j + R, :],
                            start=True,
                            stop=True,
                            tile_position=(RP * j, 0),
                        )
                    t_sb = w_p.tile([128, 4 * O], F16, tag="tsb")
                    nc.scalar.activation(
                        t_sb[:],
                        out_ps[:],
                        mybir.ActivationFunctionType.Tanh,
                    )
                    nc.vector.tensor_scalar(
                        out_sb[:, 4 * O * j : 4 * O * (j + 1)],
                        t_sb[:],
                        OUT_SCALE,
                        None,
                        op0=mybir.AluOpType.mult,
                    )
                nc.sync.dma_start(o_v[s], out_sb[:])

    nc.compile()
    return nc


def make_consts(projections: np.ndarray, control_points: np.ndarray,
                component_weights: np.ndarray):
    grid = np.linspace(-1.0, 1.0, G).astype(np.float32)
    qmat = np.zeros((128, 2 * RP), np.float32)
    for h in range(2):
        for r in range(R):
            qmat[:, h * RP + r] = projections[h * 128 : (h + 1) * 128, r % C]
    cpbn = np.zeros((128, O), np.float32)
    biasp = np.zeros((128, 1), np.float32)
    for j in range(SUPER // QUART):
        for r in range(R):
            g, c = r // C, r % C
            cpbn[RP * j + r] = -2.5 * control_points[c, g] * component_weights[c]
            biasp[RP * j + r, 0] = -grid[g]
    return qmat.astype(np.float16), cpbn.astype(BF16_NP), biasp


def make_in_maps(x: np.ndarray, projections: np.ndarray,
                 control_points: np.ndarray, component_weights: np.ndarray):
    tok = x.shape[0] * x.shape[1]
    tok_per_core = tok // N_CORES
    n_super = tok_per_core // SUPER
    qmat, cpbn, biasp = make_consts(
        np.asarray(projections), np.asarray(control_points),
        np.asarray(component_weights)
    )
    flat = x.reshape(tok, D)
    in_maps = []
    for i in range(N_CORES):
        xc = flat[i * tok_per_core : (i + 1) * tok_per_core]
        # [tok, D] -> [s, p, h, t] -> [(s p), (h t)]: per supertile s, each
        # partition p holds feature h*128+p for tokens t, 8 KiB contiguous.
        xt = (
            xc.reshape(n_super, SUPER, 2, 128)
            .transpose(0, 3, 2, 1)
            .reshape(n_super * 128, 2 * SUPER)
            .astype(np.float16)
        )
        in_maps.append(
            {"xt": np.ascontiguousarray(xt), "qmat": qmat, "cpbn": cpbn,
             "biasp": biasp}
        )
    return in_maps


def assemble_output(outs, B, S):
    """outs: per-core [n_super*128, 16*O] int8 p-major arrays -> [B, S, O] f32."""
    tok_per_core = TOK_TOTAL // N_CORES
    n_super = tok_per_core // SUPER
    parts = []
    for o in outs:
        a = np.asarray(o).reshape(n_super, 128, SUPER // CHUNK, O)
        parts.append(
            a.transpose(0, 2, 1, 3).reshape(tok_per_core, O).astype(np.float32)
        )
    return np.concatenate(parts, axis=0).reshape(B, S, O) * (1.0 / OUT_SCALE)


_NC_CACHE = {}


def kernel(x, projections, control_points, component_weights, _trace=False):
    x = np.asarray(x)
    B, S, _ = x.shape
    tok = B * S
    tok_per_core = tok // N_CORES
    key = (tok_per_core, 1)
    if key not in _NC_CACHE:
        _NC_CACHE[key] = build_nc(tok_per_core)
    nc = _NC_CACHE[key]

    in_maps = make_in_maps(x, projections, control_points, component_weights)
    res = run_bass_kernel_spmd(nc, in_maps, list(range(N_CORES)), trace=_trace)
    ret = assemble_output(
        [res.results[i]["out"] for i in range(N_CORES)], B, S
    ).astype(x.dtype, copy=False)
    if _trace:
        return ret, res
    return ret
